# revision 11
# baseline (speedup 1.0000x reference)
"""CrossBatchEmbeddingMixer on 8 trn2 NeuronCores.

Row-shard B across 8 cores (512 rows each); bf16 matmuls (fp32 is 4x slower on
the PE), f32 cosine/softmax scaling. AllGather of raw projections + inverse
norms + values overlaps with independent GEMMs; top-8 via the HW vector.max
instruction; softmax mask+exp+rowsum fused via scalar_tensor_tensor accum_out;
transposes via DMA-transpose; LayerNorms built feature-on-partition so the gate
MLP needs no activations transposes.
"""
import numpy as np
import ml_dtypes

B, H, GH = 4096, 4096, 1024
NCORES = 8
L = B // NCORES        # 512 local rows
P = 128
KT = H // P            # 32 k-tiles over H
KH = KT // 2
MB = L // P            # 4 local row blocks
NCH = H // 512         # 8 chunks of 512
K3 = 3 * H // P        # 96 k-tiles over 3H
KG = GH // P           # 8 k-tiles over GH
LN_EPS = 1e-5
NEG = -1.0e30

bf = ml_dtypes.bfloat16

_CACHE = {}


def _build():
    import concourse.bacc as bacc
    import concourse.mybir as mybir
    import concourse.tile as tile

    dt = mybir.dt
    f32, b16 = dt.float32, dt.bfloat16
    AF = mybir.ActivationFunctionType
    OP = mybir.AluOpType
    X = mybir.AxisListType.X

    nc = bacc.Bacc("TRN2", target_bir_lowering=False, debug=False,
                   num_devices=NCORES)

    hsT_d = nc.dram_tensor("hsT", [H, L], b16, kind="ExternalInput")
    hs32_d = nc.dram_tensor("hs32", [L, H], f32, kind="ExternalInput")
    WsT_d = nc.dram_tensor("WsT", [H, H], b16, kind="ExternalInput")
    WvT_d = nc.dram_tensor("WvT", [H, H], b16, kind="ExternalInput")
    W1T_d = nc.dram_tensor("W1T", [3 * H, GH], b16, kind="ExternalInput")
    W2T_d = nc.dram_tensor("W2T", [GH, H], b16, kind="ExternalInput")
    b1_d = nc.dram_tensor("b1c", [GH, 1], f32, kind="ExternalInput")
    b2_d = nc.dram_tensor("b2r", [1, H], b16, kind="ExternalInput")
    colb_d = nc.dram_tensor("colb", [1, B], b16, kind="ExternalInput")
    gh_d = nc.dram_tensor("ghp", [P, KT], f32, kind="ExternalInput")
    bh_d = nc.dram_tensor("bhp", [P, KT], f32, kind="ExternalInput")
    ga_d = nc.dram_tensor("gap", [P, KT], f32, kind="ExternalInput")
    ba_d = nc.dram_tensor("bap", [P, KT], f32, kind="ExternalInput")
    out_d = nc.dram_tensor("out", [L, H], f32, kind="ExternalOutput")

    rg = [list(range(NCORES))]

    with tile.TileContext(nc) as tc:
        with (
            tc.tile_pool(name="per", bufs=1) as per,
            tc.tile_pool(name="hot", bufs=3) as hot,
            tc.tile_pool(name="cold", bufs=1) as cold,
            tc.tile_pool(name="ps", bufs=5, space="PSUM") as ps,
            tc.tile_pool(name="psr", bufs=1, space="PSUM") as psr,
            tc.tile_pool(name="dram", bufs=1, space="DRAM") as dram,
        ):
            # persistent pools, opened in reverse order of close (LIFO)
            pF_cm = tc.tile_pool(name="pF", bufs=1); pF = pF_cm.__enter__()
            pLh_cm = tc.tile_pool(name="pLh", bufs=1); pLh = pLh_cm.__enter__()
            pA_cm = tc.tile_pool(name="pA", bufs=1); pA = pA_cm.__enter__()

            projL = dram.tile([H, L], b16)
            projA = dram.tile([NCORES * H, L], b16, addr_space="Shared")
            invnL = dram.tile([1, L], f32)
            invnA = dram.tile([NCORES, L], f32, addr_space="Shared")
            valL = dram.tile([L, H], b16)
            valA = dram.tile([B, H], b16, addr_space="Shared")
            simD = dram.tile([L, B], f32)
            crossD = dram.tile([L, H], b16)

            hsT = pA.tile([P, KT, L], b16, tag="hsT")
            nc.sync.dma_start(hsT[:], hsT_d.ap().rearrange("(k p) l -> p k l", p=P))
            projsb = pA.tile([P, KT, L], b16, tag="projsb")

            ones_row_b = per.tile([1, P], b16, tag="ones_rb")
            nc.vector.memset(ones_row_b[:], 1.0)
            ones_row_f = per.tile([1, P], f32, tag="ones_rf")
            nc.vector.memset(ones_row_f[:], 1.0)
            ones_col_b = per.tile([P, 1], b16, tag="ones_cb")
            nc.vector.memset(ones_col_b[:], 1.0)
            inv_rs = per.tile([P, MB], f32, tag="inv_rs")

            # ---------------- Phase A: proj_T + norms ----------------
            WsTr = WsT_d.ap().rearrange("(k p) o -> p k o", p=P)
            n2ps = psr.tile([1, L], f32, tag="red1")
            with tc.tile_pool(name="wsA", bufs=2) as wsA:
                for mg in range(8):      # groups of 4 o-tiles (512 cols)
                    wsbs = []
                    for kh in range(2):
                        wsb = wsA.tile([P, KH, 512], b16, tag="wsb", name=f"wsb{kh}")
                        nc.sync.dma_start(
                            wsb[:], WsTr[:, kh * KH:(kh + 1) * KH,
                                         mg * 512:(mg + 1) * 512])
                        wsbs.append(wsb)
                    accs = [ps.tile([P, 512], f32, tag="acc", name=f"acc{m}")
                            for m in range(4)]
                    for k in range(KT):
                        for m in range(4):
                            nc.tensor.matmul(
                                accs[m][:], wsbs[k // KH][:, k % KH, m * P:(m + 1) * P],
                                hsT[:, k, :], start=(k == 0), stop=(k == KT - 1))
                    for m in range(4):
                        o = mg * 4 + m
                        nc.scalar.activation(projsb[:, o, :], accs[m][:], AF.Copy)
                        sqt = hot.tile([P, 512], b16, tag="sqA")
                        nc.vector.tensor_tensor(sqt[:], projsb[:, o, :],
                                                projsb[:, o, :], op=OP.mult)
                        nc.tensor.matmul(n2ps[:], ones_col_b[:], sqt[:],
                                         start=(o == 0), stop=(o == KT - 1))
                        nc.sync.dma_start(projL[o * P:(o + 1) * P, :], projsb[:, o, :])
            nrm = cold.tile([1, L], f32, tag="nrm")
            nc.scalar.activation(nrm[:], n2ps[:], AF.Sqrt)
            nc.vector.tensor_scalar_max(nrm[:], nrm[:], 1e-12)
            invn = cold.tile([1, L], f32, tag="invn")
            nc.vector.reciprocal(invn[:], nrm[:])
            nc.sync.dma_start(invnL[:], invn[:])

            nc.gpsimd.collective_compute("AllGather", OP.bypass,
                                         ins=[projL.opt()], outs=[projA.opt()],
                                         replica_groups=rg)
            nc.gpsimd.collective_compute("AllGather", OP.bypass,
                                         ins=[invnL.opt()], outs=[invnA.opt()],
                                         replica_groups=rg)

            # ---------------- Phase B: values ----------------
            WvTr = WvT_d.ap().rearrange("(k p) o -> p k o", p=P)
            with tc.tile_pool(name="wsB", bufs=2) as wsB:
                for n in range(NCH):
                    wvbs = []
                    for kh in range(2):
                        wvb = wsB.tile([P, KH, 512], b16, tag="wvb", name=f"wvb{kh}")
                        nc.sync.dma_start(
                            wvb[:], WvTr[:, kh * KH:(kh + 1) * KH,
                                         n * 512:(n + 1) * 512])
                        wvbs.append(wvb)
                    accs = [ps.tile([P, 512], f32, tag="acc", name=f"acc{m}")
                            for m in range(4)]
                    for k in range(KT):
                        for m in range(4):
                            nc.tensor.matmul(
                                accs[m][:], hsT[:, k, m * P:(m + 1) * P],
                                wvbs[k // KH][:, k % KH, :],
                                start=(k == 0), stop=(k == KT - 1))
                    for m in range(4):
                        vsb = hot.tile([P, 512], b16, tag="vsb")
                        nc.scalar.activation(vsb[:], accs[m][:], AF.Copy)
                        nc.sync.dma_start(
                            valL[m * P:(m + 1) * P, n * 512:(n + 1) * 512], vsb[:])
            nc.gpsimd.collective_compute("AllGather", OP.bypass,
                                         ins=[valL.opt()], outs=[valA.opt()],
                                         replica_groups=rg)

            # ---------------- transposed LayerNorm helper ----------------
            def ln_transposed(src_T, gamma_d, beta_d, dst_pool, dst_tag):
                sums = psr.tile([1, L], f32, tag="red1", name="sums")
                sqs = psr.tile([1, L], f32, tag="red2", name="sqs")
                for k in range(KT):
                    nc.tensor.matmul(sums[:], ones_col_b[:], src_T[:, k, :],
                                     start=(k == 0), stop=(k == KT - 1))
                for k in range(KT):
                    sqt = hot.tile([P, L], b16, tag="sqE")
                    nc.vector.tensor_tensor(sqt[:], src_T[:, k, :], src_T[:, k, :],
                                            op=OP.mult)
                    nc.tensor.matmul(sqs[:], ones_col_b[:], sqt[:],
                                     start=(k == 0), stop=(k == KT - 1))
                mu = cold.tile([1, L], f32, tag="mu")
                nc.vector.tensor_scalar(mu[:], sums[:], 1.0 / H, None, op0=OP.mult)
                ex2 = cold.tile([1, L], f32, tag="ex2")
                nc.vector.tensor_scalar(ex2[:], sqs[:], 1.0 / H, None, op0=OP.mult)
                mu2 = cold.tile([1, L], f32, tag="mu2")
                nc.vector.tensor_tensor(mu2[:], mu[:], mu[:], op=OP.mult)
                var = cold.tile([1, L], f32, tag="var")
                nc.vector.tensor_tensor(var[:], ex2[:], mu2[:], op=OP.subtract)
                epsb = cold.tile([1, 1], f32, tag="epsb")
                nc.vector.memset(epsb[:], LN_EPS)
                sd = cold.tile([1, L], f32, tag="sd")
                nc.scalar.activation(sd[:], var[:], AF.Sqrt, bias=epsb[:])
                rstd = cold.tile([1, L], f32, tag="rstd")
                nc.vector.reciprocal(rstd[:], sd[:])
                mub_ps = ps.tile([P, L], f32, tag="acc")
                nc.tensor.matmul(mub_ps[:], ones_row_f[:], mu[:], start=True, stop=True)
                mub = cold.tile([P, L], b16, tag="mub")
                nc.scalar.activation(mub[:], mub_ps[:], AF.Copy)
                rsb_ps = ps.tile([P, L], f32, tag="acc")
                nc.tensor.matmul(rsb_ps[:], ones_row_f[:], rstd[:], start=True, stop=True)
                rsb = cold.tile([P, L], b16, tag="rsb")
                nc.scalar.activation(rsb[:], rsb_ps[:], AF.Copy)
                gam = cold.tile([P, KT], f32, tag="gam")
                nc.sync.dma_start(gam[:], gamma_d[:])
                bet = cold.tile([P, KT], f32, tag="bet")
                nc.sync.dma_start(bet[:], beta_d[:])
                dst = dst_pool.tile([P, KT, L], b16, tag=dst_tag, name=dst_tag)
                for k in range(KT):
                    t1 = hot.tile([P, L], b16, tag="lnt1")
                    nc.vector.tensor_tensor(t1[:], src_T[:, k, :], mub[:],
                                            op=OP.subtract)
                    t2 = hot.tile([P, L], b16, tag="lnt2")
                    nc.vector.tensor_tensor(t2[:], t1[:], rsb[:], op=OP.mult)
                    nc.vector.tensor_scalar(dst[:, k, :], t2[:],
                                            gam[:, k:k + 1], bet[:, k:k + 1],
                                            op0=OP.mult, op1=OP.add)
                return dst

            lnhT = ln_transposed(hsT, gh_d, bh_d, pLh, "lnhT")

            # ---------------- Phase C: sim GEMM ----------------
            projAr = projA.rearrange("(c k p) l -> c p k l", k=KT, p=P)
            with tc.tile_pool(name="wsC", bufs=2) as wsC, \
                 tc.tile_pool(name="smc", bufs=2) as smc:
                for n in range(NCH):
                    pabs = []
                    for kh in range(2):
                        pab = wsC.tile([P, KH, 512], b16, tag="pab", name=f"pab{kh}")
                        nc.sync.dma_start(pab[:], projAr[n][:, kh * KH:(kh + 1) * KH, :])
                        pabs.append(pab)
                    invj = smc.tile([1, 512], f32, tag="invj")
                    nc.sync.dma_start(invj[:], invnA[n:n + 1, :])
                    invjb_ps = ps.tile([P, 512], f32, tag="acc")
                    nc.tensor.matmul(invjb_ps[:], ones_row_f[:], invj[:],
                                     start=True, stop=True)
                    invjb = smc.tile([P, 512], f32, tag="invjb")
                    nc.vector.tensor_copy(invjb[:], invjb_ps[:])
                    colbc = smc.tile([1, 512], b16, tag="colbc")
                    nc.sync.dma_start(colbc[:], colb_d[0:1, n * 512:(n + 1) * 512])
                    for rb in range(MB):
                        acc = ps.tile([P, 512], f32, tag="acc")
                        for k in range(KT):
                            nc.tensor.matmul(
                                acc[:], projsb[:, k, rb * P:(rb + 1) * P],
                                pabs[k // KH][:, k % KH, :],
                                start=(k == 0), stop=False)
                        nc.tensor.matmul(acc[:], ones_row_b[:], colbc[:],
                                         start=False, stop=True)
                        ssb = smc.tile([P, 512], f32, tag="ssb")
                        nc.vector.tensor_tensor(ssb[:], acc[:], invjb[:], op=OP.mult)
                        nc.sync.dma_start(
                            simD[rb * P:(rb + 1) * P, n * 512:(n + 1) * 512], ssb[:])
            pA_cm.__exit__(None, None, None)

            # ---------------- Phase C epilogue: topk + softmax ----------------
            pT_cm = tc.tile_pool(name="pT", bufs=1); pT = pT_cm.__enter__()
            eT = pT.tile([P, KT, L], b16, tag="eT")
            with tc.tile_pool(name="epi", bufs=1) as epi:
                for rb in range(MB):
                    srow = epi.tile([P, B], f32, tag="srow")
                    nc.sync.dma_start(srow[:], simD[rb * P:(rb + 1) * P, :])
                    rmax = cold.tile([P, 1], f32, tag="rmax")
                    nc.vector.tensor_reduce(rmax[:], srow[:], axis=X, op=OP.max)
                    torep = cold.tile([P, 8], f32, tag="torep")
                    nc.vector.memset(torep[:], 3.0e38)
                    nc.vector.tensor_copy(torep[:, 0:1], rmax[:])
                    srm = epi.tile([P, B], f32, tag="srm")
                    nc.vector.match_replace(srm[:], torep[:], srow[:], NEG)
                    top8 = cold.tile([P, 8], f32, tag="top8")
                    nc.vector.max(top8[:], srm[:])
                    invi = cold.tile([P, 1], f32, tag="invi")
                    nc.sync.dma_start(
                        invi[:],
                        invnL[0:1, rb * P:(rb + 1) * P].rearrange("a b -> b a"))
                    bias_t = cold.tile([P, 1], f32, tag="bias_t")
                    nc.vector.tensor_scalar(bias_t[:], top8[:, 7:8], invi[:], -1.0,
                                            op0=OP.mult, op1=OP.mult)
                    y = epi.tile([P, B], f32, tag="y")
                    nc.scalar.activation(y[:], srm[:], AF.Exp,
                                         bias=bias_t[:], scale=invi[:])
                    ebf = epi.tile([P, B], b16, tag="ebf")
                    rsum = cold.tile([P, 1], f32, tag="rsum")
                    nc.vector.scalar_tensor_tensor(
                        ebf[:], srm[:], top8[:, 7:8], y[:],
                        op0=OP.is_ge, op1=OP.mult, accum_out=rsum[:])
                    rs2 = cold.tile([P, 1], f32, tag="rs2")
                    nc.vector.tensor_scalar_max(rs2[:], rsum[:], 1e-30)
                    nc.vector.reciprocal(inv_rs[:, rb:rb + 1], rs2[:])
                    nc.sync.dma_start_transpose(
                        out=eT[:, :, rb * P:(rb + 1) * P], in_=ebf[:])

            # ---------------- Phase D: cross ----------------
            valAr = valA.rearrange("(k p) o -> p k o", p=P)
            with tc.tile_pool(name="wsD", bufs=2) as wsD:
                for n in range(NCH):
                    vabs = []
                    for kh in range(2):
                        vab = wsD.tile([P, KH, 512], b16, tag="vab", name=f"vab{kh}")
                        nc.sync.dma_start(
                            vab[:], valAr[:, kh * KH:(kh + 1) * KH,
                                          n * 512:(n + 1) * 512])
                        vabs.append(vab)
                    for rb in range(MB):
                        acc = ps.tile([P, 512], f32, tag="acc")
                        for k in range(KT):
                            nc.tensor.matmul(
                                acc[:], eT[:, k, rb * P:(rb + 1) * P],
                                vabs[k // KH][:, k % KH, :],
                                start=(k == 0), stop=(k == KT - 1))
                        crc = hot.tile([P, 512], b16, tag="crc")
                        nc.scalar.activation(crc[:], acc[:], AF.Copy,
                                             scale=inv_rs[:, rb:rb + 1])
                        nc.sync.dma_start(
                            crossD[rb * P:(rb + 1) * P, n * 512:(n + 1) * 512], crc[:])
            pT_cm.__exit__(None, None, None)

            # ---------------- Phase E: ln_a transposed ----------------
            pE_cm = tc.tile_pool(name="pE", bufs=1); pE = pE_cm.__enter__()
            with tc.tile_pool(name="pCT", bufs=1) as pCT:
                crossT = pCT.tile([P, KT, L], b16, tag="crossT")
                for rb in range(MB):
                    nc.sync.dma_start_transpose(
                        out=crossT[:, :, rb * P:(rb + 1) * P],
                        in_=crossD[rb * P:(rb + 1) * P, :])
                lnaT = ln_transposed(crossT, ga_d, ba_d, pE, "lnaT")

            # ---------------- Phase F: MLP1 ----------------
            W1Tr = W1T_d.ap().rearrange("(k p) g -> p k g", p=P)
            hidT = pF.tile([P, KG, L], b16, tag="hidT")
            with tc.tile_pool(name="wsF", bufs=2) as wsF:
                for mg in range(KG):
                    w1bs = []
                    for kh in range(2):
                        w1b = wsF.tile([P, K3 // 2, P], b16, tag="w1b",
                                       name=f"w1b{kh}")
                        nc.sync.dma_start(
                            w1b[:], W1Tr[:, kh * (K3 // 2):(kh + 1) * (K3 // 2),
                                         mg * P:(mg + 1) * P])
                        w1bs.append(w1b)
                    acc = ps.tile([P, 512], f32, tag="acc")
                    for k in range(K3):
                        if k < KT:
                            rhs = lnhT[:, k, :]
                        elif k < 2 * KT:
                            rhs = lnaT[:, k - KT, :]
                        else:
                            kk = k - 2 * KT
                            pr = hot.tile([P, L], b16, tag="prod")
                            nc.vector.tensor_tensor(pr[:], lnhT[:, kk, :],
                                                    lnaT[:, kk, :], op=OP.mult)
                            rhs = pr[:]
                        nc.tensor.matmul(acc[:], w1bs[k // (K3 // 2)][:, k % (K3 // 2), :],
                                         rhs, start=(k == 0), stop=(k == K3 - 1))
                    b1s = cold.tile([P, 1], f32, tag="b1s")
                    nc.sync.dma_start(b1s[:], b1_d[mg * P:(mg + 1) * P, :])
                    nc.scalar.activation(hidT[:, mg, :], acc[:], AF.Gelu, bias=b1s[:])
            pE_cm.__exit__(None, None, None)
            pLh_cm.__exit__(None, None, None)

            # ---------------- Phase G: MLP2 + final ----------------
            W2Tr = W2T_d.ap().rearrange("(k p) o -> p k o", p=P)
            with tc.tile_pool(name="wsG", bufs=2) as wsG, \
                 tc.tile_pool(name="smg", bufs=3) as smg:
                for n in range(NCH):
                    w2b = wsG.tile([P, KG, 512], b16, tag="w2b")
                    nc.sync.dma_start(w2b[:], W2Tr[:, :, n * 512:(n + 1) * 512])
                    b2c = smg.tile([1, 512], b16, tag="b2c")
                    nc.sync.dma_start(b2c[:], b2_d[0:1, n * 512:(n + 1) * 512])
                    for rb in range(MB):
                        acc = ps.tile([P, 512], f32, tag="acc")
                        for k in range(KG):
                            nc.tensor.matmul(
                                acc[:], hidT[:, k, rb * P:(rb + 1) * P],
                                w2b[:, k, :], start=(k == 0), stop=False)
                        nc.tensor.matmul(acc[:], ones_row_b[:], b2c[:],
                                         start=False, stop=True)
                        gate = smg.tile([P, 512], b16, tag="gate")
                        nc.scalar.activation(gate[:], acc[:], AF.Sigmoid)
                        crg = smg.tile([P, 512], b16, tag="crg")
                        nc.sync.dma_start(
                            crg[:],
                            crossD[rb * P:(rb + 1) * P, n * 512:(n + 1) * 512])
                        gc = smg.tile([P, 512], f32, tag="gc")
                        nc.vector.tensor_tensor(gc[:], gate[:], crg[:], op=OP.mult)
                        hsc = smg.tile([P, 512], f32, tag="hsc")
                        nc.sync.dma_start(
                            hsc[:], hs32_d[rb * P:(rb + 1) * P, n * 512:(n + 1) * 512])
                        oc = smg.tile([P, 512], f32, tag="oc")
                        nc.vector.tensor_tensor(oc[:], gc[:], hsc[:], op=OP.add)
                        nc.sync.dma_start(
                            out_d[rb * P:(rb + 1) * P, n * 512:(n + 1) * 512], oc[:])
            pF_cm.__exit__(None, None, None)

    nc.compile()
    return nc


def _prep(inputs):
    hs = np.asarray(inputs["hidden_states"], dtype=np.float32)
    mask = np.asarray(inputs["attention_mask"])
    Ws = np.asarray(inputs["Ws"], dtype=np.float32)
    Wv = np.asarray(inputs["Wv"], dtype=np.float32)
    W1 = np.asarray(inputs["W1"], dtype=np.float32)
    W2 = np.asarray(inputs["W2"], dtype=np.float32)
    b1 = np.asarray(inputs["b1"], dtype=np.float32)
    b2 = np.asarray(inputs["b2"], dtype=np.float32)
    g_h = np.asarray(inputs["g_h"], dtype=np.float32)
    b_h = np.asarray(inputs["b_h"], dtype=np.float32)
    g_a = np.asarray(inputs["g_a"], dtype=np.float32)
    b_a = np.asarray(inputs["b_a"], dtype=np.float32)

    hsT = np.ascontiguousarray(hs.T).astype(bf)
    WsT = np.ascontiguousarray(Ws.T).astype(bf)
    WvT = np.ascontiguousarray(Wv.T).astype(bf)
    W1T = np.ascontiguousarray(W1.T).astype(bf)
    W2T = np.ascontiguousarray(W2.T).astype(bf)
    colb = np.where(mask, 0.0, NEG).astype(bf).reshape(1, B)
    b1c = b1.reshape(GH, 1)
    b2r = b2.astype(bf).reshape(1, H)

    def pcol(v):
        return np.ascontiguousarray(v.reshape(KT, P).T)

    shared = {"WsT": WsT, "WvT": WvT, "W1T": W1T, "W2T": W2T,
              "b1c": b1c, "b2r": b2r, "colb": colb,
              "ghp": pcol(g_h), "bhp": pcol(b_h),
              "gap": pcol(g_a), "bap": pcol(b_a)}
    in_maps = []
    for c in range(NCORES):
        m = dict(shared)
        m["hsT"] = np.ascontiguousarray(hsT[:, c * L:(c + 1) * L])
        m["hs32"] = np.ascontiguousarray(hs[c * L:(c + 1) * L, :])
        in_maps.append(m)
    return in_maps


def _run(inputs, trace=False):
    from concourse.bass_utils import run_bass_kernel_spmd
    if "nc" not in _CACHE:
        _CACHE["nc"] = _build()
    nc = _CACHE["nc"]
    in_maps = _prep(inputs)
    res = run_bass_kernel_spmd(nc, in_maps, list(range(NCORES)), trace=trace)
    out = np.concatenate([res.results[c]["out"] for c in range(NCORES)], axis=0)
    return out, res


def kernel(**inputs) -> np.ndarray:
    out, _ = _run(inputs, trace=False)
    return out


# revision 22
# speedup vs baseline: 42.6156x; 42.6156x over previous
"""CrossBatchEmbeddingMixer on 8 trn2 NeuronCores.

Row-shard B across 8 cores (512 rows each); bf16 matmuls (fp32 is 4x slower on
the PE), f32 cosine/softmax scaling. AllGather of raw projections + inverse
norms + values overlaps with independent GEMMs; top-8 via the HW vector.max
instruction; softmax mask+exp+rowsum fused via scalar_tensor_tensor accum_out;
transposes via DMA-transpose; LayerNorms built feature-on-partition so the gate
MLP needs no activations transposes.
"""
import numpy as np
import ml_dtypes

B, H, GH = 4096, 4096, 1024
NCORES = 8
L = B // NCORES        # 512 local rows
P = 128
KT = H // P            # 32 k-tiles over H
KH = KT // 2
MB = L // P            # 4 local row blocks
NCH = H // 512         # 8 chunks of 512
K3 = 3 * H // P        # 96 k-tiles over 3H
KG = GH // P           # 8 k-tiles over GH
LN_EPS = 1e-5
NEG = -1.0e30

bf = ml_dtypes.bfloat16

_CACHE = {}


def _build(collectives=True):
    import concourse.bacc as bacc
    import concourse.mybir as mybir
    import concourse.tile as tile

    dt = mybir.dt
    f32, b16 = dt.float32, dt.bfloat16
    AF = mybir.ActivationFunctionType
    OP = mybir.AluOpType
    X = mybir.AxisListType.X

    nc = bacc.Bacc("TRN2", target_bir_lowering=False, debug=False,
                   num_devices=NCORES)

    hsT_d = nc.dram_tensor("hsT", [H, L], b16, kind="ExternalInput")
    hs32_d = nc.dram_tensor("hs32", [L, H], f32, kind="ExternalInput")
    WsT_d = nc.dram_tensor("WsT", [H, H], b16, kind="ExternalInput")
    WvT_d = nc.dram_tensor("WvT", [H, H], b16, kind="ExternalInput")
    W1T_d = nc.dram_tensor("W1T", [3 * H, GH], b16, kind="ExternalInput")
    W2T_d = nc.dram_tensor("W2T", [GH, H], b16, kind="ExternalInput")
    b1_d = nc.dram_tensor("b1c", [GH, 1], f32, kind="ExternalInput")
    b2_d = nc.dram_tensor("b2r", [1, H], b16, kind="ExternalInput")
    colb_d = nc.dram_tensor("colb", [1, B], b16, kind="ExternalInput")
    gh_d = nc.dram_tensor("ghp", [P, KT], f32, kind="ExternalInput")
    bh_d = nc.dram_tensor("bhp", [P, KT], f32, kind="ExternalInput")
    ga_d = nc.dram_tensor("gap", [P, KT], f32, kind="ExternalInput")
    ba_d = nc.dram_tensor("bap", [P, KT], f32, kind="ExternalInput")
    out_d = nc.dram_tensor("out", [L, H], f32, kind="ExternalOutput")

    rg = [list(range(NCORES))]

    with tile.TileContext(nc) as tc:
        with (
            tc.tile_pool(name="per", bufs=1) as per,
            tc.tile_pool(name="hot", bufs=3) as hot,
            tc.tile_pool(name="cold", bufs=1) as cold,
            tc.tile_pool(name="ps", bufs=6, space="PSUM") as ps,
            tc.tile_pool(name="psr", bufs=1, space="PSUM") as psr,
            tc.tile_pool(name="dram", bufs=1, space="DRAM") as dram,
        ):
            # persistent pools, opened in reverse order of close (LIFO)
            pF_cm = tc.tile_pool(name="pF", bufs=1); pF = pF_cm.__enter__()
            pLh_cm = tc.tile_pool(name="pLh", bufs=1); pLh = pLh_cm.__enter__()
            pA_cm = tc.tile_pool(name="pA", bufs=1); pA = pA_cm.__enter__()

            projL = dram.tile([H, L], b16)
            projA = dram.tile([NCORES * H, L], b16, addr_space="Shared")
            invnL = dram.tile([1, L], f32)
            invnA = dram.tile([NCORES, L], f32, addr_space="Shared")
            valL = dram.tile([L, H], b16)
            valA = dram.tile([B, H], b16, addr_space="Shared")
            simDs = [dram.tile([P, B], f32, name=f"simD{r}") for r in range(MB)]
            crossD = dram.tile([L, H], b16)

            hsT = pA.tile([P, KT, L], b16, tag="hsT")
            hsTr_ = hsT_d.ap().rearrange("(k p) l -> p k l", p=P)
            for q in range(4):
                nc.sync.dma_start(hsT[:, q * (KT // 4):(q + 1) * (KT // 4), :],
                                  hsTr_[:, q * (KT // 4):(q + 1) * (KT // 4), :])
            projsb = pA.tile([P, KT, L], b16, tag="projsb")

            ones_row_b = per.tile([1, P], b16, tag="ones_rb")
            nc.vector.memset(ones_row_b[:], 1.0)
            ones_row_f = per.tile([1, P], f32, tag="ones_rf")
            nc.vector.memset(ones_row_f[:], 1.0)
            ones_col_b = per.tile([P, 1], b16, tag="ones_cb")
            nc.vector.memset(ones_col_b[:], 1.0)
            inv_rs = per.tile([P, MB], f32, tag="inv_rs")

            # ---------------- Phase A: proj_T + norms ----------------
            WsTr = WsT_d.ap().rearrange("(k p) o -> p k o", p=P)
            n2ps = psr.tile([1, L], f32, tag="red1")
            with tc.tile_pool(name="wsA", bufs=2) as wsA:
                for mg in range(8):      # groups of 4 o-tiles (512 cols)
                    wsbs = []
                    for kh in range(2):
                        wsb = wsA.tile([P, KH, 512], b16, tag="wsb", name=f"wsb{kh}")
                        nc.sync.dma_start(
                            wsb[:], WsTr[:, kh * KH:(kh + 1) * KH,
                                         mg * 512:(mg + 1) * 512])
                        wsbs.append(wsb)
                    accs = [ps.tile([P, 512], f32, tag="acc", name=f"acc{m}")
                            for m in range(4)]
                    for k in range(KT):
                        for m in range(4):
                            nc.tensor.matmul(
                                accs[m][:], wsbs[k // KH][:, k % KH, m * P:(m + 1) * P],
                                hsT[:, k, :], start=(k == 0), stop=(k == KT - 1))
                    for m in range(4):
                        o = mg * 4 + m
                        nc.scalar.activation(projsb[:, o, :], accs[m][:], AF.Copy)
                        sqt = hot.tile([P, 512], b16, tag="sqA")
                        nc.vector.tensor_tensor(sqt[:], projsb[:, o, :],
                                                projsb[:, o, :], op=OP.mult)
                        nc.tensor.matmul(n2ps[:], ones_col_b[:], sqt[:],
                                         start=(o == 0), stop=(o == KT - 1))
                        nc.sync.dma_start(projL[o * P:(o + 1) * P, :], projsb[:, o, :])
            nrm = cold.tile([1, L], f32, tag="nrm")
            nc.scalar.activation(nrm[:], n2ps[:], AF.Sqrt)
            nc.vector.tensor_scalar_max(nrm[:], nrm[:], 1e-12)
            invn = cold.tile([1, L], f32, tag="invn")
            nc.vector.reciprocal(invn[:], nrm[:])
            nc.sync.dma_start(invnL[:], invn[:])

            if collectives:
                nc.gpsimd.collective_compute("AllGather", OP.bypass,
                                             ins=[projL.opt()], outs=[projA.opt()],
                                             replica_groups=rg)
                nc.gpsimd.collective_compute("AllGather", OP.bypass,
                                             ins=[invnL.opt()], outs=[invnA.opt()],
                                             replica_groups=rg)
            else:
                nc.sync.dma_start(projA[0:H, :], projL[:])
                nc.sync.dma_start(invnA[0:1, :], invnL[:])

            # ---------------- Phase B: values ----------------
            WvTr = WvT_d.ap().rearrange("(k p) o -> p k o", p=P)
            with tc.tile_pool(name="wsB", bufs=2) as wsB:
                for n in range(NCH):
                    wvbs = []
                    for kh in range(2):
                        wvb = wsB.tile([P, KH, 512], b16, tag="wvb", name=f"wvb{kh}")
                        nc.sync.dma_start(
                            wvb[:], WvTr[:, kh * KH:(kh + 1) * KH,
                                         n * 512:(n + 1) * 512])
                        wvbs.append(wvb)
                    accs = [ps.tile([P, 512], f32, tag="acc", name=f"acc{m}")
                            for m in range(4)]
                    for k in range(KT):
                        for m in range(4):
                            nc.tensor.matmul(
                                accs[m][:], hsT[:, k, m * P:(m + 1) * P],
                                wvbs[k // KH][:, k % KH, :],
                                start=(k == 0), stop=(k == KT - 1))
                    for m in range(4):
                        vsb = hot.tile([P, 512], b16, tag="vsb")
                        nc.scalar.activation(vsb[:], accs[m][:], AF.Copy)
                        nc.sync.dma_start(
                            valL[m * P:(m + 1) * P, n * 512:(n + 1) * 512], vsb[:])
            if collectives:
                nc.gpsimd.collective_compute("AllGather", OP.bypass,
                                             ins=[valL.opt()], outs=[valA.opt()],
                                             replica_groups=rg)
            else:
                nc.sync.dma_start(valA[0:L, :], valL[:])

            # ---------------- transposed LayerNorm helper ----------------
            def ln_transposed(src_T, gamma_d, beta_d, dst_pool, dst_tag,
                              stats=None):
                if stats is None:
                    sums = psr.tile([1, L], f32, tag="red1", name="sums")
                    sqs = psr.tile([1, L], f32, tag="red2", name="sqs")
                    for k in range(KT):
                        nc.tensor.matmul(sums[:], ones_col_b[:], src_T[:, k, :],
                                         start=(k == 0), stop=(k == KT - 1))
                    for k in range(KT):
                        sqt = hot.tile([P, L], b16, tag="sqE")
                        nc.vector.tensor_tensor(sqt[:], src_T[:, k, :],
                                                src_T[:, k, :], op=OP.mult)
                        nc.tensor.matmul(sqs[:], ones_col_b[:], sqt[:],
                                         start=(k == 0), stop=(k == KT - 1))
                    mu = cold.tile([1, L], f32, tag="mu")
                    nc.vector.tensor_scalar(mu[:], sums[:], 1.0 / H, None, op0=OP.mult)
                    ex2 = cold.tile([1, L], f32, tag="ex2")
                    nc.vector.tensor_scalar(ex2[:], sqs[:], 1.0 / H, None, op0=OP.mult)
                    mu2 = cold.tile([1, L], f32, tag="mu2")
                    nc.vector.tensor_tensor(mu2[:], mu[:], mu[:], op=OP.mult)
                    var = cold.tile([1, L], f32, tag="var")
                    nc.vector.tensor_tensor(var[:], ex2[:], mu2[:], op=OP.subtract)
                    epsb = cold.tile([1, 1], f32, tag="epsb")
                    nc.vector.memset(epsb[:], LN_EPS)
                    sd = cold.tile([1, L], f32, tag="sd")
                    nc.scalar.activation(sd[:], var[:], AF.Sqrt, bias=epsb[:])
                    rstd = cold.tile([1, L], f32, tag="rstd")
                    nc.vector.reciprocal(rstd[:], sd[:])
                else:
                    mu, rstd = stats
                mub_ps = ps.tile([P, L], f32, tag="acc")
                nc.tensor.matmul(mub_ps[:], ones_row_f[:], mu[:], start=True, stop=True)
                mub = cold.tile([P, L], b16, tag="mub")
                nc.scalar.activation(mub[:], mub_ps[:], AF.Copy)
                rsb_ps = ps.tile([P, L], f32, tag="acc")
                nc.tensor.matmul(rsb_ps[:], ones_row_f[:], rstd[:], start=True, stop=True)
                rsb = cold.tile([P, L], b16, tag="rsb")
                nc.scalar.activation(rsb[:], rsb_ps[:], AF.Copy)
                gam = cold.tile([P, KT], f32, tag="gam")
                nc.sync.dma_start(gam[:], gamma_d[:])
                bet = cold.tile([P, KT], f32, tag="bet")
                nc.sync.dma_start(bet[:], beta_d[:])
                dst = dst_pool.tile([P, KT, L], b16, tag=dst_tag, name=dst_tag)
                for k in range(KT):
                    t1 = hot.tile([P, L], b16, tag="lnt1")
                    nc.vector.tensor_tensor(t1[:], src_T[:, k, :], mub[:],
                                            op=OP.subtract)
                    t2 = hot.tile([P, L], b16, tag="lnt2")
                    nc.vector.tensor_tensor(t2[:], t1[:], rsb[:], op=OP.mult)
                    nc.vector.tensor_scalar(dst[:, k, :], t2[:],
                                            gam[:, k:k + 1], bet[:, k:k + 1],
                                            op0=OP.mult, op1=OP.add)
                return dst

            lnhT = ln_transposed(hsT, gh_d, bh_d, pLh, "lnhT")

            # ---------------- Phase C: sim GEMM ----------------
            projAr = projA.rearrange("(c k p) l -> c p k l", k=KT, p=P)
            with tc.tile_pool(name="wsC", bufs=2) as wsC, \
                 tc.tile_pool(name="smc", bufs=2) as smc:
                for n in range(NCH):
                    pabs = []
                    for kh in range(2):
                        pab = wsC.tile([P, KH, 512], b16, tag="pab",
                                       name=f"pab{kh}")
                        nc.sync.dma_start(
                            pab[:], projAr[n][:, kh * KH:(kh + 1) * KH, :])
                        pabs.append(pab)
                    invj = smc.tile([1, 512], f32, tag="invj")
                    nc.sync.dma_start(invj[:], invnA[n:n + 1, :])
                    invjb_ps = ps.tile([P, 512], f32, tag="acc")
                    nc.tensor.matmul(invjb_ps[:], ones_row_f[:], invj[:],
                                     start=True, stop=True)
                    invjb = smc.tile([P, 512], f32, tag="invjb")
                    nc.vector.tensor_copy(invjb[:], invjb_ps[:])
                    colbc = smc.tile([1, 512], b16, tag="colbc")
                    nc.sync.dma_start(colbc[:],
                                      colb_d[0:1, n * 512:(n + 1) * 512])
                    for rb in range(MB):
                        acc = ps.tile([P, 512], f32, tag="acc")
                        for k in range(KT):
                            nc.tensor.matmul(
                                acc[:], projsb[:, k, rb * P:(rb + 1) * P],
                                pabs[k // KH][:, k % KH, :],
                                start=(k == 0), stop=False)
                        nc.tensor.matmul(acc[:], ones_row_b[:], colbc[:],
                                         start=False, stop=True)
                        ssb = smc.tile([P, 512], f32, tag="ssb")
                        nc.vector.tensor_tensor(ssb[:], acc[:], invjb[:],
                                                op=OP.mult)
                        nc.sync.dma_start(
                            simDs[rb][:, n * 512:(n + 1) * 512], ssb[:])
            pA_cm.__exit__(None, None, None)

            # ---------------- Phase C epilogue: topk + softmax ----------------
            pT_cm = tc.tile_pool(name="pT", bufs=1); pT = pT_cm.__enter__()
            eTs = [pT.tile([P, KT, P], b16, tag=f"eT{r}", name=f"eT{r}")
                   for r in range(MB)]
            with tc.tile_pool(name="epi", bufs=1) as epi:
                for rb in range(MB):
                    srow = epi.tile([P, B], f32, tag="srow", bufs=2, name="srow")
                    nc.sync.dma_start(srow[:], simDs[rb][:])
                    rmax = cold.tile([P, 1], f32, tag="rmax", name="rmax")
                    nc.vector.tensor_reduce(rmax[:], srow[:], axis=X, op=OP.max)
                    torep = cold.tile([P, 8], f32, tag="torep", name="torep")
                    nc.vector.memset(torep[:], 3.0e38)
                    nc.vector.tensor_copy(torep[:, 0:1], rmax[:])
                    srm = epi.tile([P, B], f32, tag="srm", name="srm")
                    nc.vector.match_replace(srm[:], torep[:], srow[:], NEG)
                    top8 = cold.tile([P, 8], f32, tag="top8", name="top8")
                    nc.vector.max(top8[:], srm[:])
                    invi = cold.tile([P, 1], f32, tag="invi", name="invi")
                    nc.sync.dma_start(
                        invi[:],
                        invnL[0:1, rb * P:(rb + 1) * P].rearrange("a b -> b a"))
                    bias_t = cold.tile([P, 1], f32, tag="bias_t", name="bias_t")
                    nc.vector.tensor_scalar(bias_t[:], top8[:, 7:8], invi[:], -1.0,
                                            op0=OP.mult, op1=OP.mult)
                    y = epi.tile([P, B], f32, tag="y", name="y")
                    nc.scalar.activation(y[:], srm[:], AF.Exp,
                                         bias=bias_t[:], scale=invi[:])
                    ebf = epi.tile([P, B], b16, tag="ebf", bufs=2, name="ebf")
                    rsum = cold.tile([P, 1], f32, tag="rsum", name="rsum")
                    nc.vector.scalar_tensor_tensor(
                        ebf[:], srm[:], top8[:, 7:8], y[:],
                        op0=OP.is_ge, op1=OP.mult, accum_out=rsum[:])
                    rs2 = cold.tile([P, 1], f32, tag="rs2", name="rs2")
                    nc.vector.tensor_scalar_max(rs2[:], rsum[:], 1e-30)
                    nc.vector.reciprocal(inv_rs[:, rb:rb + 1], rs2[:])
                    nc.sync.dma_start_transpose(out=eTs[rb][:], in_=ebf[:])

            # ---------------- Phase D: cross ----------------
            valAr = valA.rearrange("(k p) o -> p k o", p=P)
            with tc.tile_pool(name="wsD", bufs=2) as wsD:
                for n in range(NCH):
                    vabs = []
                    for kh in range(2):
                        vab = wsD.tile([P, KH, 512], b16, tag="vab", name=f"vab{kh}")
                        nc.sync.dma_start(
                            vab[:], valAr[:, kh * KH:(kh + 1) * KH,
                                          n * 512:(n + 1) * 512])
                        vabs.append(vab)
                    for rb in range(MB):
                        acc = ps.tile([P, 512], f32, tag="acc")
                        for k in range(KT):
                            nc.tensor.matmul(
                                acc[:], eTs[rb][:, k, :],
                                vabs[k // KH][:, k % KH, :],
                                start=(k == 0), stop=(k == KT - 1))
                        crc = hot.tile([P, 512], b16, tag="crc")
                        nc.scalar.activation(crc[:], acc[:], AF.Copy,
                                             scale=inv_rs[:, rb:rb + 1])
                        nc.sync.dma_start(
                            crossD[rb * P:(rb + 1) * P, n * 512:(n + 1) * 512], crc[:])
            pT_cm.__exit__(None, None, None)

            # ---------------- Phase E: ln_a transposed ----------------
            pE_cm = tc.tile_pool(name="pE", bufs=1); pE = pE_cm.__enter__()
            with tc.tile_pool(name="pCT", bufs=1) as pCT:
                crossT = pCT.tile([P, KT, L], b16, tag="crossT")
                for rb in range(MB):
                    nc.sync.dma_start_transpose(
                        out=crossT[:, :, rb * P:(rb + 1) * P],
                        in_=crossD[rb * P:(rb + 1) * P, :])
                lnaT = ln_transposed(crossT, ga_d, ba_d, pE, "lnaT")

            # ---------------- Phase F: MLP1 ----------------
            W1Tr = W1T_d.ap().rearrange("(k p) g -> p k g", p=P)
            hidT = pF.tile([P, KG, L], b16, tag="hidT")
            with tc.tile_pool(name="wsF", bufs=2) as wsF:
                for mg in range(KG):
                    w1bs = []
                    for kh in range(2):
                        w1b = wsF.tile([P, K3 // 2, P], b16, tag="w1b",
                                       name=f"w1b{kh}")
                        nc.sync.dma_start(
                            w1b[:], W1Tr[:, kh * (K3 // 2):(kh + 1) * (K3 // 2),
                                         mg * P:(mg + 1) * P])
                        w1bs.append(w1b)
                    acc = ps.tile([P, 512], f32, tag="acc")
                    for k in range(K3):
                        if k < KT:
                            rhs = lnhT[:, k, :]
                        elif k < 2 * KT:
                            rhs = lnaT[:, k - KT, :]
                        else:
                            kk = k - 2 * KT
                            pr = hot.tile([P, L], b16, tag="prod")
                            nc.vector.tensor_tensor(pr[:], lnhT[:, kk, :],
                                                    lnaT[:, kk, :], op=OP.mult)
                            rhs = pr[:]
                        nc.tensor.matmul(acc[:], w1bs[k // (K3 // 2)][:, k % (K3 // 2), :],
                                         rhs, start=(k == 0), stop=(k == K3 - 1))
                    b1s = cold.tile([P, 1], f32, tag="b1s")
                    nc.sync.dma_start(b1s[:], b1_d[mg * P:(mg + 1) * P, :])
                    nc.scalar.activation(hidT[:, mg, :], acc[:], AF.Gelu, bias=b1s[:])
            pE_cm.__exit__(None, None, None)
            pLh_cm.__exit__(None, None, None)

            # ---------------- Phase G: MLP2 + final ----------------
            W2Tr = W2T_d.ap().rearrange("(k p) o -> p k o", p=P)
            with tc.tile_pool(name="wsG", bufs=2) as wsG, \
                 tc.tile_pool(name="smg", bufs=3) as smg:
                for n in range(NCH):
                    w2b = wsG.tile([P, KG, 512], b16, tag="w2b")
                    nc.sync.dma_start(w2b[:], W2Tr[:, :, n * 512:(n + 1) * 512])
                    b2c = smg.tile([1, 512], b16, tag="b2c")
                    nc.sync.dma_start(b2c[:], b2_d[0:1, n * 512:(n + 1) * 512])
                    for rb in range(MB):
                        acc = ps.tile([P, 512], f32, tag="acc")
                        for k in range(KG):
                            nc.tensor.matmul(
                                acc[:], hidT[:, k, rb * P:(rb + 1) * P],
                                w2b[:, k, :], start=(k == 0), stop=False)
                        nc.tensor.matmul(acc[:], ones_row_b[:], b2c[:],
                                         start=False, stop=True)
                        gate = smg.tile([P, 512], b16, tag="gate")
                        nc.scalar.activation(gate[:], acc[:], AF.Sigmoid)
                        crg = smg.tile([P, 512], b16, tag="crg")
                        nc.sync.dma_start(
                            crg[:],
                            crossD[rb * P:(rb + 1) * P, n * 512:(n + 1) * 512])
                        gc = smg.tile([P, 512], f32, tag="gc")
                        nc.vector.tensor_tensor(gc[:], gate[:], crg[:], op=OP.mult)
                        hsc = smg.tile([P, 512], f32, tag="hsc")
                        nc.sync.dma_start(
                            hsc[:], hs32_d[rb * P:(rb + 1) * P, n * 512:(n + 1) * 512])
                        oc = smg.tile([P, 512], f32, tag="oc")
                        nc.vector.tensor_tensor(oc[:], gc[:], hsc[:], op=OP.add)
                        nc.sync.dma_start(
                            out_d[rb * P:(rb + 1) * P, n * 512:(n + 1) * 512], oc[:])
            pF_cm.__exit__(None, None, None)

    nc.compile()
    return nc


def _prep(inputs):
    hs = np.asarray(inputs["hidden_states"], dtype=np.float32)
    mask = np.asarray(inputs["attention_mask"])
    Ws = np.asarray(inputs["Ws"], dtype=np.float32)
    Wv = np.asarray(inputs["Wv"], dtype=np.float32)
    W1 = np.asarray(inputs["W1"], dtype=np.float32)
    W2 = np.asarray(inputs["W2"], dtype=np.float32)
    b1 = np.asarray(inputs["b1"], dtype=np.float32)
    b2 = np.asarray(inputs["b2"], dtype=np.float32)
    g_h = np.asarray(inputs["g_h"], dtype=np.float32)
    b_h = np.asarray(inputs["b_h"], dtype=np.float32)
    g_a = np.asarray(inputs["g_a"], dtype=np.float32)
    b_a = np.asarray(inputs["b_a"], dtype=np.float32)

    hsT = np.ascontiguousarray(hs.T).astype(bf)
    WsT = np.ascontiguousarray(Ws.T).astype(bf)
    WvT = np.ascontiguousarray(Wv.T).astype(bf)
    W1T = np.ascontiguousarray(W1.T).astype(bf)
    W2T = np.ascontiguousarray(W2.T).astype(bf)
    colb = np.where(mask, 0.0, NEG).astype(bf).reshape(1, B)
    b1c = b1.reshape(GH, 1)
    b2r = b2.astype(bf).reshape(1, H)

    def pcol(v):
        return np.ascontiguousarray(v.reshape(KT, P).T)

    shared = {"WsT": WsT, "WvT": WvT, "W1T": W1T, "W2T": W2T,
              "b1c": b1c, "b2r": b2r, "colb": colb,
              "ghp": pcol(g_h), "bhp": pcol(b_h),
              "gap": pcol(g_a), "bap": pcol(b_a)}
    in_maps = []
    for c in range(NCORES):
        m = dict(shared)
        m["hsT"] = np.ascontiguousarray(hsT[:, c * L:(c + 1) * L])
        m["hs32"] = np.ascontiguousarray(hs[c * L:(c + 1) * L, :])
        in_maps.append(m)
    return in_maps


def _run(inputs, trace=False):
    from concourse.bass_utils import run_bass_kernel_spmd
    if "nc" not in _CACHE:
        _CACHE["nc"] = _build()
    nc = _CACHE["nc"]
    in_maps = _prep(inputs)
    res = run_bass_kernel_spmd(nc, in_maps, list(range(NCORES)), trace=trace)
    out = np.concatenate([res.results[c]["out"] for c in range(NCORES)], axis=0)
    return out, res


def kernel(**inputs) -> np.ndarray:
    out, _ = _run(inputs, trace=False)
    return out
